# revision 7
# baseline (speedup 1.0000x reference)
"""MoE layer (shared SwiGLU expert + 8 routed SwiGLU experts, sigmoid top-2
routing) on 8 Trainium2 NeuronCores.

Sharding strategy (expert-parallel, per the problem's sharding hint):
  - Router (x @ Wr, sigmoid, top-k, gate normalization) and the token->expert
    dispatch run on host as part of input sharding: core e receives exactly the
    tokens routed to expert e (capacity-padded so all 8 cores share one SPMD
    program), plus a 1/8 token shard for the replicated shared expert.
  - Each core holds only its own expert's weights (Wg[e], Wu[e], Wd[e]) plus
    the shared-expert weights, and computes two SwiGLU FFNs:
        routed: [C, D] tokens  -> silu(X Wg) * (X Wu) @ Wd
        shared: [T/8, D] shard -> same with shared weights
  - Host applies the top-k combine weights and scatter-adds expert outputs
    back to token positions (the "all-to-all return"), then adds the shared
    output.

Device kernel details:
  - All activations live transposed ([D, tokens]) so both FFN matmuls use the
    natural weight layout as the stationary operand and no on-device
    transposes are needed.
  - bf16 inputs/weights, fp32 PSUM accumulation, fp32 outputs.
  - Weights and activations are fully SBUF-resident; DMAs are split per
    128-row slice to spread across DMA engines and overlap with compute.
"""

import numpy as np
import ml_dtypes

import concourse.bass as bass
import concourse.mybir as mybir
import concourse.tile as tile
from concourse.bass_utils import run_bass_kernel_spmd

B, L, D, F, E = 2, 2048, 1024, 1024, 8
NCORES = 8
P = 128  # SBUF partitions
KB = D // P  # k-blocks per contraction (8)
NT = 512  # free-dim tile (one fp32 PSUM bank)

_BF16 = mybir.dt.bfloat16
_F32 = mybir.dt.float32

_nc_cache = {}


def _ctiles(c):
    """Split c columns into tiles of <= NT."""
    out = []
    c0 = 0
    while c0 < c:
        out.append((c0, min(NT, c - c0)))
        c0 += NT
    return out


def build_bass(C, TS):
    """One SPMD program: two SwiGLU FFNs (routed capacity C, shared shard TS)."""
    from contextlib import ExitStack

    nc = bass.Bass()
    xt = nc.declare_dram_parameter("xt", [D, C], _BF16, isOutput=False)
    xst = nc.declare_dram_parameter("xst", [D, TS], _BF16, isOutput=False)
    wg = nc.declare_dram_parameter("wg", [D, F], _BF16, isOutput=False)
    wu = nc.declare_dram_parameter("wu", [D, F], _BF16, isOutput=False)
    wd = nc.declare_dram_parameter("wd", [F, D], _BF16, isOutput=False)
    wgs = nc.declare_dram_parameter("wgs", [D, F], _BF16, isOutput=False)
    wus = nc.declare_dram_parameter("wus", [D, F], _BF16, isOutput=False)
    wds = nc.declare_dram_parameter("wds", [F, D], _BF16, isOutput=False)
    yr = nc.declare_dram_parameter("yr", [D, C], _F32, isOutput=True)
    ys = nc.declare_dram_parameter("ys", [D, TS], _F32, isOutput=True)

    with tile.TileContext(nc) as tc, ExitStack() as ctx:
        res = ctx.enter_context(tc.tile_pool(name="resident", bufs=1))
        psum = ctx.enter_context(tc.tile_pool(name="psum", bufs=2, space="PSUM"))
        tmp = ctx.enter_context(tc.tile_pool(name="tmp", bufs=3))
        outp = ctx.enter_context(tc.tile_pool(name="outp", bufs=3))

        def load_kpn(dram_ap, name):
            # [K*P, N] dram -> [P, K, N] sbuf, one DMA per k-slice
            n = dram_ap.shape[1]
            t = res.tile([P, KB, n], _BF16, tag=name)
            r = dram_ap.rearrange("(k p) n -> p k n", p=P)
            for k in range(KB):
                nc.sync.dma_start(out=t[:, k, :], in_=r[:, k, :])
            return t

        # Load order = consumption order so the first matmuls start early.
        wg_sb = load_kpn(wg, "wg")
        xt_sb = load_kpn(xt, "xt")
        wu_sb = load_kpn(wu, "wu")
        wd_sb = load_kpn(wd, "wd")
        wgs_sb = load_kpn(wgs, "wgs")
        xst_sb = load_kpn(xst, "xst")
        wus_sb = load_kpn(wus, "wus")
        wds_sb = load_kpn(wds, "wds")

        def ffn(x_sb, c, wg_t, wu_t, wd_t, y_dram, name):
            cts = _ctiles(c)
            s_sb = res.tile([P, KB, c], _BF16, tag=f"s_{name}")
            # stage A: s = silu(x Wg) * (x Wu), computed transposed per
            # (f-block m, column tile)
            for m in range(KB):
                ms = slice(m * P, (m + 1) * P)
                for c0, cn in cts:
                    cs = slice(c0, c0 + cn)
                    pg = psum.tile([P, NT], _F32, tag="pg")
                    pu = psum.tile([P, NT], _F32, tag="pu")
                    for k in range(KB):
                        nc.tensor.matmul(
                            pg[:, :cn],
                            wg_t[:, k, ms],
                            x_sb[:, k, cs],
                            start=(k == 0),
                            stop=(k == KB - 1),
                        )
                    for k in range(KB):
                        nc.tensor.matmul(
                            pu[:, :cn],
                            wu_t[:, k, ms],
                            x_sb[:, k, cs],
                            start=(k == 0),
                            stop=(k == KB - 1),
                        )
                    # silu(hg) * hu == (hg * hu) * sigmoid(hg). Sequenced so no
                    # instruction needs >1 cross-engine sync wait (TRN2 TT
                    # limit) and no instruction reads two PSUM operands:
                    #   copy pu->SBUF   (DVE waits PE; tick covers pg too)
                    #   sigmoid(pg)     (ACT waits PE)
                    #   pg * tu         (DVE, PSUM+SBUF, no new waits)
                    #   t * sg          (DVE waits ACT)
                    tu = tmp.tile([P, NT], _F32, tag="tu")
                    nc.vector.tensor_copy(tu[:, :cn], pu[:, :cn])
                    sg = tmp.tile([P, NT], _F32, tag="sg")
                    nc.scalar.activation(
                        out=sg[:, :cn],
                        in_=pg[:, :cn],
                        func=mybir.ActivationFunctionType.Sigmoid,
                    )
                    t = tmp.tile([P, NT], _F32, tag="t")
                    nc.vector.tensor_mul(t[:, :cn], pg[:, :cn], tu[:, :cn])
                    nc.vector.tensor_mul(s_sb[:, m, cs], t[:, :cn], sg[:, :cn])
            # stage B: y = s @ Wd (transposed)
            yre = y_dram.rearrange("(m p) c -> p m c", p=P)
            for m in range(KB):
                ms = slice(m * P, (m + 1) * P)
                for c0, cn in cts:
                    cs = slice(c0, c0 + cn)
                    po = psum.tile([P, NT], _F32, tag="po")
                    for k in range(KB):
                        nc.tensor.matmul(
                            po[:, :cn],
                            wd_t[:, k, ms],
                            s_sb[:, k, cs],
                            start=(k == 0),
                            stop=(k == KB - 1),
                        )
                    ot = outp.tile([P, NT], _F32, tag="ot")
                    nc.vector.tensor_copy(ot[:, :cn], po[:, :cn])
                    nc.sync.dma_start(out=yre[:, m, cs], in_=ot[:, :cn])

        ffn(xt_sb, C, wg_sb, wu_sb, wd_sb, yr, "r")
        ffn(xst_sb, TS, wgs_sb, wus_sb, wds_sb, ys, "s")

    _split_multi_waits(nc)
    return nc


def _split_multi_waits(nc):
    """The bundled walrus lowers at most ONE sync wait per instruction (every
    instruction struct has a single EVENTS slot and codegen refuses to split).
    Tile emits multi-wait sync_infos, so hoist all but one wait onto
    InstEventSemaphore carriers inserted just before the instruction on the
    same engine queue — the sequencer blocks on the carriers first, which is
    strictly more conservative than the original multi-wait semantics."""
    f = nc.m.functions[0]
    for bb in f.blocks:
        insts = bb.instructions
        idx = 0
        while idx < len(insts):
            ins = insts[idx]
            si = ins.sync_info
            if si is not None and len(si.on_wait) > 1:
                waits = list(si.on_wait)
                keep = len(waits) - 1
                if isinstance(ins, mybir.InstDMACopy):
                    for j, w in enumerate(waits):
                        if w.ant_name and w.ant_name.startswith("DMA"):
                            keep = j
                            break
                carriers = []
                for j, w in enumerate(waits):
                    if j == keep:
                        continue
                    es = mybir.InstEventSemaphore(
                        name=nc.get_next_instruction_name(), ins=[], outs=[]
                    )
                    es.engine = ins.engine
                    es.sync_info = mybir.SyncInfo(on_wait=[w], on_update=[])
                    nc.register_instruction(es)
                    carriers.append(es)
                ins.sync_info = mybir.SyncInfo(
                    on_wait=[waits[keep]], on_update=list(si.on_update)
                )
                for c in reversed(carriers):
                    insts.insert(idx, c)
                idx += len(carriers)
            idx += 1


def route(xf, Wr, expert_bias, k):
    """Host router: replicates the reference routing math exactly (fp32)."""
    logits = xf @ Wr + expert_bias[None, :]
    gates = 1.0 / (1.0 + np.exp(-logits))
    # stable argsort matches jax.lax.top_k tie-breaking (lowest index first)
    order = np.argsort(-gates, axis=1, kind="stable")
    topk_idx = order[:, :k]
    topk_gates = np.take_along_axis(gates, topk_idx, axis=1)
    topk_gates = topk_gates / (topk_gates.sum(axis=1, keepdims=True) + 1e-9)
    return topk_idx, topk_gates


def prepare(x, Wg_s, Wu_s, Wd_s, Wg, Wu, Wd, Wr, expert_bias, top_k):
    """Host-side sharding: routing + per-expert gather + weight distribution."""
    bf16 = ml_dtypes.bfloat16
    x = np.asarray(x, np.float32)
    xf = x.reshape(-1, D)
    T = xf.shape[0]
    TS = T // NCORES
    k = int(top_k)

    topk_idx, topk_gates = route(
        xf, np.asarray(Wr, np.float32), np.asarray(expert_bias, np.float32), k
    )

    idx_e, w_e = [], []
    for e in range(E):
        mask = topk_idx == e
        rows = np.nonzero(mask.any(axis=1))[0]
        idx_e.append(rows)
        w_e.append((mask[rows] * topk_gates[rows]).sum(axis=1).astype(np.float32))
    n_e = [len(r) for r in idx_e]
    C = max(P, -(-max(n_e) // P) * P)

    shared_w = {
        "wgs": np.asarray(Wg_s, np.float32).astype(bf16),
        "wus": np.asarray(Wu_s, np.float32).astype(bf16),
        "wds": np.asarray(Wd_s, np.float32).astype(bf16),
    }
    in_maps = []
    for e in range(E):
        xe = np.zeros((D, C), bf16)
        xe[:, : n_e[e]] = xf[idx_e[e]].T.astype(bf16)
        in_maps.append(
            {
                "xt": xe,
                "xst": np.ascontiguousarray(xf[e * TS : (e + 1) * TS].T).astype(bf16),
                "wg": np.asarray(Wg[e], np.float32).astype(bf16),
                "wu": np.asarray(Wu[e], np.float32).astype(bf16),
                "wd": np.asarray(Wd[e], np.float32).astype(bf16),
                **shared_w,
            }
        )
    return in_maps, idx_e, w_e, C, TS, x.shape


def combine(results, idx_e, w_e, out_shape):
    """Host-side unshard: weighted scatter-add of expert outputs + shared."""
    T = out_shape[0] * out_shape[1]
    out = np.zeros((T, D), np.float32)
    TS = T // NCORES
    for e in range(E):
        n = len(idx_e[e])
        out[idx_e[e]] += results[e]["yr"][:, :n].T * w_e[e][:, None]
        out[e * TS : (e + 1) * TS] += results[e]["ys"].T
    return out.reshape(out_shape)


def run_spmd(in_maps, C, TS, **kwargs):
    key = (C, TS)
    if key not in _nc_cache:
        _nc_cache[key] = build_bass(C, TS)
    return run_bass_kernel_spmd(
        _nc_cache[key], in_maps, core_ids=list(range(NCORES)), **kwargs
    )


def kernel(x, Wg_s, Wu_s, Wd_s, Wg, Wu, Wd, Wr, expert_bias, top_k):
    in_maps, idx_e, w_e, C, TS, out_shape = prepare(
        x, Wg_s, Wu_s, Wd_s, Wg, Wu, Wd, Wr, expert_bias, top_k
    )
    results = run_spmd(in_maps, C, TS).results
    out = combine(results, idx_e, w_e, out_shape)
    aux_loss = np.zeros((), dtype=np.float32)
    return out, aux_loss
